# revision 9
# baseline (speedup 1.0000x reference)
"""Trainium2 Bass kernel for the gated equivariant tensor-product layer.

Math (per node z, MUL=64):
  x0 = feats[:, :64], x1[u,i] = feats[:, 64+3u+i], a0 = attrs[:,0], a1 = attrs[:,1:4]
  out0 = ALPHA*( (x0*a0) @ W1 + C*(sum_i x1_i*a1_i) @ W2 )          # [N,128] = s|g
  out1_i = ALPHA*C*( (x0*a1_i) @ W3 + (x1_i*a0) @ W4 )              # [N,64] per i
  out = [ silu(s) | sigmoid(g)[w]*out1_i[w] at col 64+3w+i ]

Design (v2, feature-major):
 - HOST pre-transposes the inputs to feature-major fp16 layouts so the
   kernel needs NO PE transposes and no psum->sbuf operand staging:
     featsT [384, n]: rows [x0 | x1_0 | x0 | x1_1 | x0 | x1_2]
     attrsT [4, n]:   rows [a0 | a1_0 | a1_1 | a1_2]
   Host also reassembles the fp16 feature-major output (layout only; all
   arithmetic stays on-device).
 - Per-node attr scalars are replicated across partitions by tiny
   "replication matmuls": sel[4,128]^T @ attrsT -> psum fp16 A-mix tiles,
   copied to SBUF so the staging multiplies run in DVE 2x mode.
 - 6 staging multiplies/chunk produce pre-scaled product tiles paired so
   every product matmul is a contract-128 chain:
     S1=[t0;dt_0] S2=[dt_1;dt_2] S3=[t3_0;t4_0] S4=[t3_1;t4_1] S5=[t3_2;t4_2]
 - Products (per 512-half, f32 psum banks): B1=[s;g], B2=[o1_0;o1_1],
   B3=[o1_2;s_dup]; sigmoids on ACT; gating multiplies write fp16 SBUF
   out tiles which DMA straight to DRAM.

Sharding: pure data parallelism over nodes, 8 cores x 25000 nodes
(padded to 25600 = 25 chunks of 1024 per core).
"""

import sys
import numpy as np

sys.path.insert(0, "/opt/trn_rl_repo")

MUL = 64
C3 = 1.0 / np.sqrt(3.0)
ALPHA = 1.0 / np.sqrt(MUL * 1 * 2)

N_CORES = 8
N_PER = 25000
N_PAD = 25600
CHUNK = 1024
N_CHUNKS = N_PAD // CHUNK
P = 128

_BUILT = None


def _build_nc():
    import concourse.bacc as bacc
    import concourse.mybir as mybir
    from concourse.tile import TileContext

    f32 = mybir.dt.float32
    f16 = mybir.dt.float16
    MULT = mybir.AluOpType.mult
    AF = mybir.ActivationFunctionType

    nc = bacc.Bacc("TRN2", target_bir_lowering=False, debug=False)

    featsT_d = nc.declare_dram_parameter("featsT", [384, N_PAD], f16, isOutput=False)
    attrsT_d = nc.declare_dram_parameter("attrsT", [4, N_PAD], f16, isOutput=False)
    selc_d = nc.declare_dram_parameter("selc", [4, 640], f16, isOutput=False)
    w1_d = nc.declare_dram_parameter("W1", [64, 128], f32, isOutput=False)
    w2_d = nc.declare_dram_parameter("W2", [64, 128], f32, isOutput=False)
    w3_d = nc.declare_dram_parameter("W3", [64, 64], f32, isOutput=False)
    w4_d = nc.declare_dram_parameter("W4", [64, 64], f32, isOutput=False)
    outT_d = nc.declare_dram_parameter("outT", [256, N_PAD], f16, isOutput=True)

    with TileContext(nc) as tc:
        wpool = tc.alloc_tile_pool(name="wpool", bufs=1)
        ft = tc.alloc_tile_pool(name="ft", bufs=3)
        asb = tc.alloc_tile_pool(name="asb", bufs=2)
        st = tc.alloc_tile_pool(name="st", bufs=2)
        usb = tc.alloc_tile_pool(name="usb", bufs=2)
        ot = tc.alloc_tile_pool(name="ot", bufs=3)
        ps_a = tc.alloc_tile_pool(name="ps_a", bufs=1, space="PSUM")
        ps_b = tc.alloc_tile_pool(name="ps_b", bufs=2, space="PSUM")

        # --- weights / constants (once) ---
        wtmp = wpool.tile([P, 128], f32, tag="wtmp")
        nc.sync.dma_start(wtmp[0:64, :], w1_d[:, :])
        nc.sync.dma_start(wtmp[64:128, :], w2_d[:, :])
        nc.vector.tensor_scalar_mul(wtmp[0:64, :], wtmp[0:64, :], float(ALPHA))
        nc.vector.tensor_scalar_mul(wtmp[64:128, :], wtmp[64:128, :], float(ALPHA * C3))
        Wc0 = wpool.tile([P, 128], f16, tag="Wc0")
        Wc4 = wpool.tile([P, 128], f16, tag="Wc4")
        nc.vector.tensor_copy(Wc0[:, :], wtmp[:, :])
        nc.scalar.copy(Wc4[0:64, :], wtmp[64:128, :])
        nc.scalar.copy(Wc4[64:128, :], wtmp[64:128, :])

        wtmp2 = wpool.tile([P, 64], f32, tag="wtmp2")
        nc.sync.dma_start(wtmp2[0:64, :], w3_d[:, :])
        nc.sync.dma_start(wtmp2[64:128, :], w4_d[:, :])
        nc.vector.tensor_scalar_mul(wtmp2[:, :], wtmp2[:, :], float(ALPHA * C3))
        LA = wpool.tile([P, 64], f16, tag="LA")
        nc.vector.tensor_copy(LA[:, :], wtmp2[:, :])
        LA2 = wpool.tile([P, 64], f16, tag="LA2")
        nc.scalar.copy(LA2[0:64, :], wtmp2[64:128, :])
        nc.scalar.copy(LA2[64:128, :], wtmp2[0:64, :])

        # selector columns for the 4 A-mix replication matmuls
        selc = wpool.tile([4, 5, P], f16, tag="selc")
        nc.sync.dma_start(selc[:], selc_d[:, :].rearrange("c (m p) -> c m p", p=P))

        # whole-core attrsT resident in SBUF
        AT = wpool.tile([4, N_PAD], f16, tag="AT")
        nc.sync.dma_start(AT[:], attrsT_d[:, :])

        # Per chunk:
        #   A-mixes m: 0=[a0;a1_0] 1=[a1_0;a0] 2=[a1_1;a0] 3=[a1_2;a0]
        def prep(ch):
            z0 = ch * CHUNK
            F = ft.tile([P, 3, CHUNK], f16, tag="F")
            nc.sync.dma_start(
                F[:], featsT_d[:, z0 : z0 + CHUNK].rearrange("(t p) n -> p t n", p=P)
            )
            # A-mix replication: per-half 1-bank psum tiles in a 2-buf pool
            # so the tiny REP matmuls stream on PE without waiting for the
            # psum->sbuf casts (PE queue is in-order; any REP stall blocks
            # the product matmuls queued behind it and drops PE's p-state).
            A = asb.tile([P, 5, CHUNK], f16, tag="A")
            for m in range(5):
                for h in range(2):
                    hz = slice(z0 + h * 512, z0 + (h + 1) * 512)
                    ha = slice(h * 512, (h + 1) * 512)
                    Aps = ps_a.tile([P, 512], f32, tag="Apsum")
                    nc.tensor.matmul(Aps[:, :], selc[:, m], AT[:, hz])
                    if (m + h) % 2 == 0:
                        nc.vector.tensor_copy(A[:, m, ha], Aps[:, :])
                    else:
                        nc.scalar.copy(A[:, m, ha], Aps[:, :])

            # staging products (S tiles paired for contract-128 matmuls);
            # every op keeps in/out partition ranges identical (no DVE
            # cross-partition movement).  F tiles: F0=[x0;x1_0] F1=[x1_1;x0]
            # F2=[x0;x1_2]
            S = st.tile([P, 5, CHUNK], f16, tag="S")
            # S1 = [t0; dt_0] = F0 o [a0; a1_0]
            nc.vector.tensor_tensor(S[:, 0], F[:, 0], A[:, 0], MULT)
            # S3 = [t3_0; t4_0] = F0 o [a1_0; a0]
            nc.vector.tensor_tensor(S[:, 2], F[:, 0], A[:, 1], MULT)
            # S4 = [t4_1; t3_1] = F1 o [a0; a1_1]   (lhsT LA2 swaps W3/W4)
            nc.gpsimd.tensor_tensor(S[:, 3], F[:, 1], A[:, 2], MULT)
            # S5 = [t3_2; t4_2] = F2 o [a1_2; a0]
            nc.gpsimd.tensor_tensor(S[:, 4], F[:, 2], A[:, 3], MULT)
            # S2 = [dt_1; dt_2]: dt_1 = x1_1*a1_1 (all at partitions 0:64),
            # dt_2 = x1_2*a1_2 (all at partitions 64:128)
            nc.vector.tensor_tensor(S[0:64, 1], F[0:64, 1], A[0:64, 4], MULT)
            nc.vector.tensor_tensor(S[64:128, 1], F[64:128, 2], A[64:128, 4], MULT)
            return S

        def crunch(ch, S):
            z0 = ch * CHUNK
            OT = ot.tile([P, 2, CHUNK], f16, tag="OT")
            for h in range(2):
                hs = slice(h * 512, (h + 1) * 512)
                B1 = ps_b.tile([P, 512], f32, tag="B1")  # [s; g]
                B2 = ps_b.tile([P, 512], f32, tag="B2")  # [o1_0; o1_1]
                B3 = ps_b.tile([P, 512], f32, tag="B3")  # [o1_2; s_dup]
                nc.tensor.matmul(B1[:, :], Wc0[:, :], S[:, 0, hs], start=True, stop=False)
                nc.tensor.matmul(B1[:, :], Wc4[:, :], S[:, 1, hs], start=False, stop=True)
                nc.tensor.matmul(B2[0:64, :], LA[:, :], S[:, 2, hs])
                nc.tensor.matmul(B2[64:128, :], LA2[:, :], S[:, 3, hs])
                nc.tensor.matmul(B3[0:64, :], LA[:, :], S[:, 4, hs])
                nc.tensor.matmul(
                    B3[64:128, :], Wc0[:, 0:64], S[:, 0, hs], start=True, stop=False
                )
                nc.tensor.matmul(
                    B3[64:128, :], Wc4[:, 0:64], S[:, 1, hs], start=False, stop=True
                )

                # U2 = [sg; sg], U3 = [sg; ss]
                U2 = usb.tile([P, 512], f16, tag="U2")
                U3 = usb.tile([P, 512], f16, tag="U3")
                nc.scalar.activation(U2[0:64, :], B1[64:128, :], AF.Sigmoid)
                nc.scalar.copy(U2[64:128, :], U2[0:64, :])
                nc.scalar.activation(U3[64:128, :], B3[64:128, :], AF.Sigmoid)
                nc.vector.tensor_copy(U3[0:64, :], U2[0:64, :])

                # gating: OT[:,0]=[g0;g1], OT[:,1]=[g2;silu]
                nc.vector.tensor_tensor(OT[:, 0, hs], B2[:, :], U2[:, :], MULT)
                nc.vector.tensor_tensor(OT[:, 1, hs], B3[:, :], U3[:, :], MULT)

            nc.sync.dma_start(
                outT_d[:, z0 : z0 + CHUNK].rearrange("(t p) n -> p t n", p=P),
                OT[:],
            )

        pend = {}
        for ch in range(N_CHUNKS + 1):
            if ch < N_CHUNKS:
                pend[ch] = prep(ch)
            if ch - 1 in pend:
                crunch(ch - 1, pend.pop(ch - 1))

        for pool in (ps_b, ps_a, ot, usb, st, asb, ft, wpool):
            pool.release()

    nc.compile()
    return nc


def _get_nc():
    global _BUILT
    if _BUILT is None:
        _BUILT = _build_nc()
    return _BUILT


def _host_prep(node_feats, node_attrs):
    """Feature-major fp16 layouts per core (layout/dtype only, no math)."""
    feats = np.ascontiguousarray(node_feats, dtype=np.float32)
    attrs = np.ascontiguousarray(node_attrs, dtype=np.float32)
    in_maps = []
    for c in range(N_CORES):
        f = feats[c * N_PER : (c + 1) * N_PER]
        a = attrs[c * N_PER : (c + 1) * N_PER]
        n = f.shape[0]
        x0 = f[:, :MUL]
        x1 = f[:, MUL:].reshape(n, MUL, 3)
        ftT = np.zeros((384, N_PAD), np.float16)
        ftT[0:64, :n] = x0.T
        ftT[64:128, :n] = x1[:, :, 0].T
        ftT[128:192, :n] = x1[:, :, 1].T
        ftT[192:256, :n] = x0.T
        ftT[256:320, :n] = x0.T
        ftT[320:384, :n] = x1[:, :, 2].T
        atT = np.zeros((4, N_PAD), np.float16)
        atT[0, :n] = a[:, 0]
        atT[1, :n] = a[:, 1]
        atT[2, :n] = a[:, 2]
        atT[3, :n] = a[:, 3]
        in_maps.append({"featsT": ftT, "attrsT": atT})
    return in_maps


def _sel_const():
    # selc[c, m*128+p] = 1 where mix m partition p reads attr row c
    # mixes: 0=[a0;a1_0] 1=[a1_0;a0] 2=[a0;a1_1] 3=[a1_2;a0] 4=[a1_1;a1_2]
    sel = np.zeros((4, 640), np.float16)
    mixes = [(0, 1), (1, 0), (0, 2), (3, 0), (2, 3)]
    for m, (top, bot) in enumerate(mixes):
        sel[top, m * 128 : m * 128 + 64] = 1.0
        sel[bot, m * 128 + 64 : m * 128 + 128] = 1.0
    return sel


def kernel(node_feats, node_attrs, W1, W2, W3, W4):
    from concourse.bass_utils import run_bass_kernel_spmd

    nc = _get_nc()
    in_maps = _host_prep(node_feats, node_attrs)
    sel = _sel_const()
    for im in in_maps:
        im["selc"] = sel
        im["W1"] = np.ascontiguousarray(W1, np.float32)
        im["W2"] = np.ascontiguousarray(W2, np.float32)
        im["W3"] = np.ascontiguousarray(W3, np.float32)
        im["W4"] = np.ascontiguousarray(W4, np.float32)

    res = run_bass_kernel_spmd(nc, in_maps, list(range(N_CORES)))
    global LAST_RESULT
    LAST_RESULT = res

    outs = []
    for c in range(N_CORES):
        oT = res.results[c]["outT"][:, :N_PER].astype(np.float32)  # [256, n]
        n = oT.shape[1]
        out = np.empty((n, 256), np.float32)
        out[:, :MUL] = oT[192:256, :].T  # silu(s)
        # gated rows: [g0(0:64) | g1(64:128) | g2(128:192)], col 64+3w+i
        out[:, MUL:] = oT[0:192, :].reshape(3, 64, n).transpose(2, 1, 0).reshape(n, 192)
        outs.append(out)
    return np.concatenate(outs, axis=0)


LAST_RESULT = None


# revision 10
# speedup vs baseline: 1.7485x; 1.7485x over previous
"""Trainium2 Bass kernel for the gated equivariant tensor-product layer.

Math (per node z, MUL=64):
  x0 = feats[:, :64], x1[u,i] = feats[:, 64+3u+i], a0 = attrs[:,0], a1 = attrs[:,1:4]
  out0 = ALPHA*( (x0*a0) @ W1 + C*(sum_i x1_i*a1_i) @ W2 )          # [N,128] = s|g
  out1_i = ALPHA*C*( (x0*a1_i) @ W3 + (x1_i*a0) @ W4 )              # [N,64] per i
  out = [ silu(s) | sigmoid(g)[w]*out1_i[w] at col 64+3w+i ]

Design (v4, feature-major, host-staged layouts):
 - HOST pre-transposes inputs to feature-major fp16 (layout/dtype only, no
   arithmetic) and pre-REPLICATES the per-node attr scalars into four
   128-row "mix" tiles so the kernel has zero transposes, zero psum->sbuf
   operand casts, and the tensor engine runs nothing but dense product
   matmuls (keeps it at full p-state clock):
     featsT [256, n]: rows [x0 | x1_0 | x1_1 | x1_2]
     amixT [512, n]:  rows [a0;a1_0 | a1_0;a0 | a1_1;a1_2 | a1_2;a0]
 - 7 staging multiplies/chunk (DVE 2x-mode / GPSIMD) produce product tiles
   paired so every product matmul is a contract-128 chain:
     S1=[t0;dt_0] S2=[dt_1;dt_2] S3=[t3_0;t4_0] S4=[t3_1;t4_1] S5=[t3_2;t4_2]
 - Products in f32 psum [128,1024] 2-bank tiles (matmuls write 512-halves);
   sigmoids on ACT; gating multiplies write fp16 SBUF out tiles -> DMA.
 - Host reassembles the fp16 feature-major output.

Sharding: pure data parallelism over nodes, 8 cores x 25000 nodes
(padded to 25600 = 25 chunks of 1024 per core).
"""

import sys
import numpy as np

sys.path.insert(0, "/opt/trn_rl_repo")

MUL = 64
C3 = 1.0 / np.sqrt(3.0)
ALPHA = 1.0 / np.sqrt(MUL * 1 * 2)

N_CORES = 8
N_PER = 25000
N_PAD = 25600
CHUNK = 1024
N_CHUNKS = N_PAD // CHUNK
P = 128

_BUILT = None


def _build_nc():
    import concourse.bacc as bacc
    import concourse.mybir as mybir
    from concourse.tile import TileContext

    f32 = mybir.dt.float32
    f16 = mybir.dt.float16
    MULT = mybir.AluOpType.mult
    AF = mybir.ActivationFunctionType

    nc = bacc.Bacc("TRN2", target_bir_lowering=False, debug=False)

    featsT_d = nc.declare_dram_parameter("featsT", [256, N_PAD], f16, isOutput=False)
    amixT_d = nc.declare_dram_parameter("amixT", [512, N_PAD], f16, isOutput=False)
    w1_d = nc.declare_dram_parameter("W1", [64, 128], f32, isOutput=False)
    w2_d = nc.declare_dram_parameter("W2", [64, 128], f32, isOutput=False)
    w3_d = nc.declare_dram_parameter("W3", [64, 64], f32, isOutput=False)
    w4_d = nc.declare_dram_parameter("W4", [64, 64], f32, isOutput=False)
    outT_d = nc.declare_dram_parameter("outT", [256, N_PAD], f16, isOutput=True)

    with TileContext(nc) as tc:
        wpool = tc.alloc_tile_pool(name="wpool", bufs=1)
        ft = tc.alloc_tile_pool(name="ft", bufs=3)
        am = tc.alloc_tile_pool(name="am", bufs=3)
        st = tc.alloc_tile_pool(name="st", bufs=2)
        usb = tc.alloc_tile_pool(name="usb", bufs=2)
        ot = tc.alloc_tile_pool(name="ot", bufs=3)
        ps_b = tc.alloc_tile_pool(name="ps_b", bufs=1, space="PSUM")

        # --- weights (once) ---
        wtmp = wpool.tile([P, 128], f32, tag="wtmp")
        nc.sync.dma_start(wtmp[0:64, :], w1_d[:, :])
        nc.sync.dma_start(wtmp[64:128, :], w2_d[:, :])
        nc.vector.tensor_scalar_mul(wtmp[0:64, :], wtmp[0:64, :], float(ALPHA))
        nc.vector.tensor_scalar_mul(wtmp[64:128, :], wtmp[64:128, :], float(ALPHA * C3))
        Wc0 = wpool.tile([P, 128], f16, tag="Wc0")
        Wc4 = wpool.tile([P, 128], f16, tag="Wc4")
        nc.vector.tensor_copy(Wc0[:, :], wtmp[:, :])
        nc.scalar.copy(Wc4[0:64, :], wtmp[64:128, :])
        nc.scalar.copy(Wc4[64:128, :], wtmp[64:128, :])

        wtmp2 = wpool.tile([P, 64], f32, tag="wtmp2")
        nc.sync.dma_start(wtmp2[0:64, :], w3_d[:, :])
        nc.sync.dma_start(wtmp2[64:128, :], w4_d[:, :])
        nc.vector.tensor_scalar_mul(wtmp2[:, :], wtmp2[:, :], float(ALPHA * C3))
        LA = wpool.tile([P, 64], f16, tag="LA")
        nc.vector.tensor_copy(LA[:, :], wtmp2[:, :])

        def prep(ch):
            z0 = ch * CHUNK
            # T1 = F[:,0] = [x0; x1_0], T2 = F[:,1] = [x1_1; x1_2]
            F = ft.tile([P, 2, CHUNK], f16, tag="F")
            nc.sync.dma_start(
                F[:], featsT_d[:, z0 : z0 + CHUNK].rearrange("(t p) n -> p t n", p=P)
            )
            # M[:,0]=[a0;a1_0] M[:,1]=[a1_0;a0] M[:,2]=[a1_1;a1_2] M[:,3]=[a1_2;a0]
            M = am.tile([P, 4, CHUNK], f16, tag="M")
            nc.sync.dma_start(
                M[:], amixT_d[:, z0 : z0 + CHUNK].rearrange("(t p) n -> p t n", p=P)
            )

            S = st.tile([P, 5, CHUNK], f16, tag="S")
            # S1 = [t0; dt_0]
            nc.vector.tensor_tensor(S[:, 0], F[:, 0], M[:, 0], MULT)
            # S2 = [dt_1; dt_2]
            nc.gpsimd.tensor_tensor(S[:, 1], F[:, 1], M[:, 2], MULT)
            # S3 = [t3_0; t4_0]
            nc.gpsimd.tensor_tensor(S[:, 2], F[:, 0], M[:, 1], MULT)
            # S4 = [t3_1; t4_1]: t3_1 = x0*a1_1; t4_1 = x1_1*a0 (out-base 64)
            nc.vector.tensor_tensor(S[0:64, 3], F[0:64, 0], M[0:64, 2], MULT)
            nc.vector.tensor_tensor(S[64:128, 3], F[0:64, 1], M[0:64, 0], MULT)
            # S5 = [t3_2; t4_2]: t3_2 = x0*a1_2; t4_2 = x1_2*a0
            nc.vector.tensor_tensor(S[0:64, 4], F[0:64, 0], M[0:64, 3], MULT)
            nc.vector.tensor_tensor(S[64:128, 4], F[64:128, 1], M[64:128, 3], MULT)
            return S

        def crunch(ch, S):
            z0 = ch * CHUNK
            OT = ot.tile([P, 2, CHUNK], f16, tag="OT")
            B1 = ps_b.tile([P, CHUNK], f32, tag="B1")  # [s; g]
            B2 = ps_b.tile([P, CHUNK], f32, tag="B2")  # [o1_0; o1_1]
            B3 = ps_b.tile([P, CHUNK], f32, tag="B3")  # [o1_2; s_dup]
            for h in range(2):
                hs = slice(h * 512, (h + 1) * 512)
                nc.tensor.matmul(B1[:, hs], Wc0[:, :], S[:, 0, hs], start=True, stop=False)
                nc.tensor.matmul(B1[:, hs], Wc4[:, :], S[:, 1, hs], start=False, stop=True)
                nc.tensor.matmul(B2[0:64, hs], LA[:, :], S[:, 2, hs])
                nc.tensor.matmul(B2[64:128, hs], LA[:, :], S[:, 3, hs])
                nc.tensor.matmul(B3[0:64, hs], LA[:, :], S[:, 4, hs])
                nc.tensor.matmul(
                    B3[64:128, hs], Wc0[:, 0:64], S[:, 0, hs], start=True, stop=False
                )
                nc.tensor.matmul(
                    B3[64:128, hs], Wc4[:, 0:64], S[:, 1, hs], start=False, stop=True
                )

            # U2 = [sg; sg], U3 = [sg; ss]
            U2 = usb.tile([P, CHUNK], f16, tag="U2")
            U3 = usb.tile([P, CHUNK], f16, tag="U3")
            nc.scalar.activation(U2[0:64, :], B1[64:128, :], AF.Sigmoid)
            nc.scalar.copy(U2[64:128, :], U2[0:64, :])
            nc.scalar.activation(U3[64:128, :], B3[64:128, :], AF.Sigmoid)
            nc.vector.tensor_copy(U3[0:64, :], U2[0:64, :])

            # gating: OT[:,0]=[g0;g1], OT[:,1]=[g2;silu]
            nc.vector.tensor_tensor(OT[:, 0], B2[:, :], U2[:, :], MULT)
            nc.vector.tensor_tensor(OT[:, 1], B3[:, :], U3[:, :], MULT)

            nc.sync.dma_start(
                outT_d[:, z0 : z0 + CHUNK].rearrange("(t p) n -> p t n", p=P),
                OT[:],
            )

        pend = {}
        for ch in range(N_CHUNKS + 1):
            if ch < N_CHUNKS:
                pend[ch] = prep(ch)
            if ch - 1 in pend:
                crunch(ch - 1, pend.pop(ch - 1))

        for pool in (ps_b, ot, usb, st, am, ft, wpool):
            pool.release()

    nc.compile()
    return nc


def _get_nc():
    global _BUILT
    if _BUILT is None:
        _BUILT = _build_nc()
    return _BUILT


def _host_prep(node_feats, node_attrs):
    """Feature-major fp16 layouts per core (layout/dtype/replication only)."""
    feats = np.ascontiguousarray(node_feats, dtype=np.float32)
    attrs = np.ascontiguousarray(node_attrs, dtype=np.float32)
    in_maps = []
    for c in range(N_CORES):
        f = feats[c * N_PER : (c + 1) * N_PER]
        a = attrs[c * N_PER : (c + 1) * N_PER].astype(np.float16)
        n = f.shape[0]
        x0 = f[:, :MUL]
        x1 = f[:, MUL:].reshape(n, MUL, 3)
        ftT = np.zeros((256, N_PAD), np.float16)
        ftT[0:64, :n] = x0.T
        ftT[64:128, :n] = x1[:, :, 0].T
        ftT[128:192, :n] = x1[:, :, 1].T
        ftT[192:256, :n] = x1[:, :, 2].T
        amT = np.zeros((512, N_PAD), np.float16)
        a0, a10, a11, a12 = a[:, 0], a[:, 1], a[:, 2], a[:, 3]
        for r0, src in (
            (0, a0), (64, a10),      # M1 = [a0; a1_0]
            (128, a10), (192, a0),   # M2 = [a1_0; a0]
            (256, a11), (320, a12),  # M3 = [a1_1; a1_2]
            (384, a12), (448, a0),   # M4 = [a1_2; a0]
        ):
            amT[r0 : r0 + 64, :n] = src[None, :]
        in_maps.append({"featsT": ftT, "amixT": amT})
    return in_maps


def kernel(node_feats, node_attrs, W1, W2, W3, W4):
    from concourse.bass_utils import run_bass_kernel_spmd

    nc = _get_nc()
    in_maps = _host_prep(node_feats, node_attrs)
    for im in in_maps:
        im["W1"] = np.ascontiguousarray(W1, np.float32)
        im["W2"] = np.ascontiguousarray(W2, np.float32)
        im["W3"] = np.ascontiguousarray(W3, np.float32)
        im["W4"] = np.ascontiguousarray(W4, np.float32)

    res = run_bass_kernel_spmd(nc, in_maps, list(range(N_CORES)))
    global LAST_RESULT
    LAST_RESULT = res

    outs = []
    for c in range(N_CORES):
        oT = res.results[c]["outT"][:, :N_PER].astype(np.float32)  # [256, n]
        n = oT.shape[1]
        out = np.empty((n, 256), np.float32)
        out[:, :MUL] = oT[192:256, :].T  # silu(s)
        # gated rows: [g0(0:64) | g1(64:128) | g2(128:192)], col 64+3w+i
        out[:, MUL:] = oT[0:192, :].reshape(3, 64, n).transpose(2, 1, 0).reshape(n, 192)
        outs.append(out)
    return np.concatenate(outs, axis=0)


LAST_RESULT = None


# revision 11
# speedup vs baseline: 1.9515x; 1.1161x over previous
"""Trainium2 Bass kernel for the gated equivariant tensor-product layer.

Math (per node z, MUL=64):
  x0 = feats[:, :64], x1[u,i] = feats[:, 64+3u+i], a0 = attrs[:,0], a1 = attrs[:,1:4]
  out0 = ALPHA*( (x0*a0) @ W1 + C*(sum_i x1_i*a1_i) @ W2 )          # [N,128] = s|g
  out1_i = ALPHA*C*( (x0*a1_i) @ W3 + (x1_i*a0) @ W4 )              # [N,64] per i
  out = [ silu(s) | sigmoid(g)[w]*out1_i[w] at col 64+3w+i ]

Design (v4, feature-major, host-staged layouts):
 - HOST pre-transposes inputs to feature-major fp16 (layout/dtype only, no
   arithmetic) and pre-REPLICATES the per-node attr scalars into four
   128-row "mix" tiles so the kernel has zero transposes, zero psum->sbuf
   operand casts, and the tensor engine runs nothing but dense product
   matmuls (keeps it at full p-state clock):
     featsT [256, n]: rows [x0 | x1_0 | x1_1 | x1_2]
     amixT [512, n]:  rows [a0;a1_0 | a1_0;a0 | a1_1;a1_2 | a1_2;a0]
 - 7 staging multiplies/chunk (DVE 2x-mode / GPSIMD) produce product tiles
   paired so every product matmul is a contract-128 chain:
     S1=[t0;dt_0] S2=[dt_1;dt_2] S3=[t3_0;t4_0] S4=[t3_1;t4_1] S5=[t3_2;t4_2]
 - Products in f32 psum [128,1024] 2-bank tiles (matmuls write 512-halves);
   sigmoids on ACT; gating multiplies write fp16 SBUF out tiles -> DMA.
 - Host reassembles the fp16 feature-major output.

Sharding: pure data parallelism over nodes, 8 cores x 25000 nodes
(padded to 25600 = 25 chunks of 1024 per core).
"""

import sys
import numpy as np

sys.path.insert(0, "/opt/trn_rl_repo")

MUL = 64
C3 = 1.0 / np.sqrt(3.0)
ALPHA = 1.0 / np.sqrt(MUL * 1 * 2)

N_CORES = 8
N_PER = 25000
N_PAD = 25600
CHUNK = 1024
N_CHUNKS = N_PAD // CHUNK
P = 128

_BUILT = None


def _build_nc():
    import concourse.bacc as bacc
    import concourse.mybir as mybir
    from concourse.tile import TileContext

    f32 = mybir.dt.float32
    f16 = mybir.dt.float16
    MULT = mybir.AluOpType.mult
    AF = mybir.ActivationFunctionType

    nc = bacc.Bacc("TRN2", target_bir_lowering=False, debug=False)

    featsT_d = nc.declare_dram_parameter("featsT", [384, N_PAD], f16, isOutput=False)
    amixT_d = nc.declare_dram_parameter("amixT", [512, N_PAD], f16, isOutput=False)
    w1_d = nc.declare_dram_parameter("W1", [64, 128], f32, isOutput=False)
    w2_d = nc.declare_dram_parameter("W2", [64, 128], f32, isOutput=False)
    w3_d = nc.declare_dram_parameter("W3", [64, 64], f32, isOutput=False)
    w4_d = nc.declare_dram_parameter("W4", [64, 64], f32, isOutput=False)
    outT_d = nc.declare_dram_parameter("outT", [256, N_PAD], f16, isOutput=True)

    with TileContext(nc) as tc:
        wpool = tc.alloc_tile_pool(name="wpool", bufs=1)
        ft = tc.alloc_tile_pool(name="ft", bufs=3)
        am = tc.alloc_tile_pool(name="am", bufs=3)
        st = tc.alloc_tile_pool(name="st", bufs=2)
        usb = tc.alloc_tile_pool(name="usb", bufs=2)
        ot = tc.alloc_tile_pool(name="ot", bufs=3)
        ps_b = tc.alloc_tile_pool(name="ps_b", bufs=1, space="PSUM")

        # --- weights (once) ---
        wtmp = wpool.tile([P, 128], f32, tag="wtmp")
        nc.sync.dma_start(wtmp[0:64, :], w1_d[:, :])
        nc.sync.dma_start(wtmp[64:128, :], w2_d[:, :])
        nc.vector.tensor_scalar_mul(wtmp[0:64, :], wtmp[0:64, :], float(ALPHA))
        nc.vector.tensor_scalar_mul(wtmp[64:128, :], wtmp[64:128, :], float(ALPHA * C3))
        Wc0 = wpool.tile([P, 128], f16, tag="Wc0")
        Wc4 = wpool.tile([P, 128], f16, tag="Wc4")
        nc.vector.tensor_copy(Wc0[:, :], wtmp[:, :])
        nc.scalar.copy(Wc4[0:64, :], wtmp[64:128, :])
        nc.scalar.copy(Wc4[64:128, :], wtmp[64:128, :])

        wtmp2 = wpool.tile([P, 64], f32, tag="wtmp2")
        nc.sync.dma_start(wtmp2[0:64, :], w3_d[:, :])
        nc.sync.dma_start(wtmp2[64:128, :], w4_d[:, :])
        nc.vector.tensor_scalar_mul(wtmp2[:, :], wtmp2[:, :], float(ALPHA * C3))
        LA = wpool.tile([P, 64], f16, tag="LA")
        nc.vector.tensor_copy(LA[:, :], wtmp2[:, :])
        LA2 = wpool.tile([P, 64], f16, tag="LA2")
        nc.scalar.copy(LA2[0:64, :], wtmp2[64:128, :])
        nc.scalar.copy(LA2[64:128, :], wtmp2[0:64, :])

        def prep(ch):
            z0 = ch * CHUNK
            # T1 = F[:,0] = [x0; x1_0], T2 = F[:,1] = [x1_1; x1_2],
            # T3 = F[:,2] = [x1_1; x0]
            F = ft.tile([P, 3, CHUNK], f16, tag="F")
            nc.sync.dma_start(
                F[:], featsT_d[:, z0 : z0 + CHUNK].rearrange("(t p) n -> p t n", p=P)
            )
            # M[:,0]=[a0;a1_0] M[:,1]=[a1_0;a0] M[:,2]=[a1_1;a1_2] M[:,3]=[a0;a1_1]
            M = am.tile([P, 4, CHUNK], f16, tag="M")
            nc.sync.dma_start(
                M[:], amixT_d[:, z0 : z0 + CHUNK].rearrange("(t p) n -> p t n", p=P)
            )

            S = st.tile([P, 5, CHUNK], f16, tag="S")
            # S1 = [t0; dt_0]
            nc.vector.tensor_tensor(S[:, 0], F[:, 0], M[:, 0], MULT)
            # S2 = [dt_1; dt_2]
            nc.gpsimd.tensor_tensor(S[:, 1], F[:, 1], M[:, 2], MULT)
            # S3 = [t3_0; t4_0]
            nc.gpsimd.tensor_tensor(S[:, 2], F[:, 0], M[:, 1], MULT)
            # S4' = [t4_1; t3_1] = [x1_1; x0] o [a0; a1_1]  (lhsT LA2)
            nc.vector.tensor_tensor(S[:, 3], F[:, 2], M[:, 3], MULT)
            # S5 = [t3_2; t4_2]: t3_2 = x0*a1_2 (out-base flex), t4_2 = x1_2*a0
            nc.vector.tensor_tensor(S[0:64, 4], F[64:128, 2], M[64:128, 2], MULT)
            nc.vector.tensor_tensor(S[64:128, 4], F[64:128, 1], M[64:128, 1], MULT)
            return S

        def crunch(ch, S):
            z0 = ch * CHUNK
            OT = ot.tile([P, 2, CHUNK], f16, tag="OT")
            B1 = ps_b.tile([P, CHUNK], f32, tag="B1")  # [s; g]
            B2 = ps_b.tile([P, CHUNK], f32, tag="B2")  # [o1_0; o1_1]
            B3 = ps_b.tile([P, CHUNK], f32, tag="B3")  # [o1_2; s_dup]
            for h in range(2):
                hs = slice(h * 512, (h + 1) * 512)
                nc.tensor.matmul(B1[:, hs], Wc0[:, :], S[:, 0, hs], start=True, stop=False)
                nc.tensor.matmul(B1[:, hs], Wc4[:, :], S[:, 1, hs], start=False, stop=True)
                nc.tensor.matmul(B2[0:64, hs], LA[:, :], S[:, 2, hs])
                nc.tensor.matmul(B2[64:128, hs], LA2[:, :], S[:, 3, hs])
                nc.tensor.matmul(B3[0:64, hs], LA[:, :], S[:, 4, hs])
                nc.tensor.matmul(
                    B3[64:128, hs], Wc0[:, 0:64], S[:, 0, hs], start=True, stop=False
                )
                nc.tensor.matmul(
                    B3[64:128, hs], Wc4[:, 0:64], S[:, 1, hs], start=False, stop=True
                )

            # U2 = [sg; sg], U3 = [sg; ss]
            U2 = usb.tile([P, CHUNK], f16, tag="U2")
            U3 = usb.tile([P, CHUNK], f16, tag="U3")
            nc.scalar.activation(U2[0:64, :], B1[64:128, :], AF.Sigmoid)
            nc.scalar.copy(U2[64:128, :], U2[0:64, :])
            nc.scalar.activation(U3[64:128, :], B3[64:128, :], AF.Sigmoid)
            nc.scalar.copy(U3[0:64, :], U2[0:64, :])

            # gating: OT[:,0]=[g0;g1], OT[:,1]=[g2;silu]
            nc.vector.tensor_tensor(OT[:, 0], B2[:, :], U2[:, :], MULT)
            nc.vector.tensor_tensor(OT[:, 1], B3[:, :], U3[:, :], MULT)

            nc.sync.dma_start(
                outT_d[:, z0 : z0 + CHUNK].rearrange("(t p) n -> p t n", p=P),
                OT[:],
            )

        pend = {}
        for ch in range(N_CHUNKS + 1):
            if ch < N_CHUNKS:
                pend[ch] = prep(ch)
            if ch - 1 in pend:
                crunch(ch - 1, pend.pop(ch - 1))

        for pool in (ps_b, ot, usb, st, am, ft, wpool):
            pool.release()

    nc.compile()
    return nc


def _get_nc():
    global _BUILT
    if _BUILT is None:
        _BUILT = _build_nc()
    return _BUILT


def _host_prep(node_feats, node_attrs):
    """Feature-major fp16 layouts per core (layout/dtype/replication only)."""
    feats = np.ascontiguousarray(node_feats, dtype=np.float32)
    attrs = np.ascontiguousarray(node_attrs, dtype=np.float32)
    in_maps = []
    for c in range(N_CORES):
        f = feats[c * N_PER : (c + 1) * N_PER]
        a = attrs[c * N_PER : (c + 1) * N_PER].astype(np.float16)
        n = f.shape[0]
        x0 = f[:, :MUL]
        x1 = f[:, MUL:].reshape(n, MUL, 3)
        ftT = np.zeros((384, N_PAD), np.float16)
        ftT[0:64, :n] = x0.T
        ftT[64:128, :n] = x1[:, :, 0].T
        ftT[128:192, :n] = x1[:, :, 1].T
        ftT[192:256, :n] = x1[:, :, 2].T
        ftT[256:320, :n] = x1[:, :, 1].T
        ftT[320:384, :n] = x0.T
        amT = np.zeros((512, N_PAD), np.float16)
        a0, a10, a11, a12 = a[:, 0], a[:, 1], a[:, 2], a[:, 3]
        for r0, src in (
            (0, a0), (64, a10),      # M1 = [a0; a1_0]
            (128, a10), (192, a0),   # M2 = [a1_0; a0]
            (256, a11), (320, a12),  # M3 = [a1_1; a1_2]
            (384, a0), (448, a11),   # M4 = [a0; a1_1]
        ):
            amT[r0 : r0 + 64, :n] = src[None, :]
        in_maps.append({"featsT": ftT, "amixT": amT})
    return in_maps


def kernel(node_feats, node_attrs, W1, W2, W3, W4):
    from concourse.bass_utils import run_bass_kernel_spmd

    nc = _get_nc()
    in_maps = _host_prep(node_feats, node_attrs)
    for im in in_maps:
        im["W1"] = np.ascontiguousarray(W1, np.float32)
        im["W2"] = np.ascontiguousarray(W2, np.float32)
        im["W3"] = np.ascontiguousarray(W3, np.float32)
        im["W4"] = np.ascontiguousarray(W4, np.float32)

    res = run_bass_kernel_spmd(nc, in_maps, list(range(N_CORES)))
    global LAST_RESULT
    LAST_RESULT = res

    outs = []
    for c in range(N_CORES):
        oT = res.results[c]["outT"][:, :N_PER].astype(np.float32)  # [256, n]
        n = oT.shape[1]
        out = np.empty((n, 256), np.float32)
        out[:, :MUL] = oT[192:256, :].T  # silu(s)
        # gated rows: [g0(0:64) | g1(64:128) | g2(128:192)], col 64+3w+i
        out[:, MUL:] = oT[0:192, :].reshape(3, 64, n).transpose(2, 1, 0).reshape(n, 192)
        outs.append(out)
    return np.concatenate(outs, axis=0)


LAST_RESULT = None


# revision 12
# speedup vs baseline: 1.9686x; 1.0088x over previous
"""Trainium2 Bass kernel for the gated equivariant tensor-product layer.

Math (per node z, MUL=64):
  x0 = feats[:, :64], x1[u,i] = feats[:, 64+3u+i], a0 = attrs[:,0], a1 = attrs[:,1:4]
  out0 = ALPHA*( (x0*a0) @ W1 + C*(sum_i x1_i*a1_i) @ W2 )          # [N,128] = s|g
  out1_i = ALPHA*C*( (x0*a1_i) @ W3 + (x1_i*a0) @ W4 )              # [N,64] per i
  out = [ silu(s) | sigmoid(g)[w]*out1_i[w] at col 64+3w+i ]

Design (v4, feature-major, host-staged layouts):
 - HOST pre-transposes inputs to feature-major fp16 (layout/dtype only, no
   arithmetic) and pre-REPLICATES the per-node attr scalars into four
   128-row "mix" tiles so the kernel has zero transposes, zero psum->sbuf
   operand casts, and the tensor engine runs nothing but dense product
   matmuls (keeps it at full p-state clock):
     featsT [256, n]: rows [x0 | x1_0 | x1_1 | x1_2]
     amixT [512, n]:  rows [a0;a1_0 | a1_0;a0 | a1_1;a1_2 | a1_2;a0]
 - 7 staging multiplies/chunk (DVE 2x-mode / GPSIMD) produce product tiles
   paired so every product matmul is a contract-128 chain:
     S1=[t0;dt_0] S2=[dt_1;dt_2] S3=[t3_0;t4_0] S4=[t3_1;t4_1] S5=[t3_2;t4_2]
 - Products in f32 psum [128,1024] 2-bank tiles (matmuls write 512-halves);
   sigmoids on ACT; gating multiplies write fp16 SBUF out tiles -> DMA.
 - Host reassembles the fp16 feature-major output.

Sharding: pure data parallelism over nodes, 8 cores x 25000 nodes
(padded to 25600 = 25 chunks of 1024 per core).
"""

import sys
import numpy as np

sys.path.insert(0, "/opt/trn_rl_repo")

MUL = 64
C3 = 1.0 / np.sqrt(3.0)
ALPHA = 1.0 / np.sqrt(MUL * 1 * 2)

N_CORES = 8
N_PER = 25000
N_PAD = 25600
CHUNK = 1024
N_CHUNKS = N_PAD // CHUNK
P = 128

_BUILT = None


def _build_nc():
    import concourse.bacc as bacc
    import concourse.mybir as mybir
    from concourse.tile import TileContext

    f32 = mybir.dt.float32
    f16 = mybir.dt.float16
    MULT = mybir.AluOpType.mult
    AF = mybir.ActivationFunctionType

    nc = bacc.Bacc("TRN2", target_bir_lowering=False, debug=False)

    featsT_d = nc.declare_dram_parameter("featsT", [384, N_PAD], f16, isOutput=False)
    amixT_d = nc.declare_dram_parameter("amixT", [512, N_PAD], f16, isOutput=False)
    w1_d = nc.declare_dram_parameter("W1", [64, 128], f32, isOutput=False)
    w2_d = nc.declare_dram_parameter("W2", [64, 128], f32, isOutput=False)
    w3_d = nc.declare_dram_parameter("W3", [64, 64], f32, isOutput=False)
    w4_d = nc.declare_dram_parameter("W4", [64, 64], f32, isOutput=False)
    outT_d = nc.declare_dram_parameter("outT", [256, N_PAD], f16, isOutput=True)

    with TileContext(nc) as tc:
        wpool = tc.alloc_tile_pool(name="wpool", bufs=1)
        ft = tc.alloc_tile_pool(name="ft", bufs=3)
        am = tc.alloc_tile_pool(name="am", bufs=3)
        st = tc.alloc_tile_pool(name="st", bufs=2)
        usb = tc.alloc_tile_pool(name="usb", bufs=2)
        ot = tc.alloc_tile_pool(name="ot", bufs=3)
        ps_b = tc.alloc_tile_pool(name="ps_b", bufs=1, space="PSUM")

        # --- weights (once) ---
        wtmp = wpool.tile([P, 128], f32, tag="wtmp")
        nc.sync.dma_start(wtmp[0:64, :], w1_d[:, :])
        nc.sync.dma_start(wtmp[64:128, :], w2_d[:, :])
        nc.vector.tensor_scalar_mul(wtmp[0:64, :], wtmp[0:64, :], float(ALPHA))
        nc.vector.tensor_scalar_mul(wtmp[64:128, :], wtmp[64:128, :], float(ALPHA * C3))
        Wc0 = wpool.tile([P, 128], f16, tag="Wc0")
        Wc4 = wpool.tile([P, 128], f16, tag="Wc4")
        nc.vector.tensor_copy(Wc0[:, :], wtmp[:, :])
        nc.scalar.copy(Wc4[0:64, :], wtmp[64:128, :])
        nc.scalar.copy(Wc4[64:128, :], wtmp[64:128, :])

        wtmp2 = wpool.tile([P, 64], f32, tag="wtmp2")
        nc.sync.dma_start(wtmp2[0:64, :], w3_d[:, :])
        nc.sync.dma_start(wtmp2[64:128, :], w4_d[:, :])
        nc.vector.tensor_scalar_mul(wtmp2[:, :], wtmp2[:, :], float(ALPHA * C3))
        LA = wpool.tile([P, 64], f16, tag="LA")
        nc.vector.tensor_copy(LA[:, :], wtmp2[:, :])
        LA2 = wpool.tile([P, 64], f16, tag="LA2")
        nc.scalar.copy(LA2[0:64, :], wtmp2[64:128, :])
        nc.scalar.copy(LA2[64:128, :], wtmp2[0:64, :])

        def prep(ch):
            z0 = ch * CHUNK
            # T1 = F[:,0] = [x0; x1_0], T2 = F[:,1] = [x1_1; x1_2],
            # T3 = F[:,2] = [x1_1; x0]
            F = ft.tile([P, 3, CHUNK], f16, tag="F")
            nc.sync.dma_start(
                F[:], featsT_d[:, z0 : z0 + CHUNK].rearrange("(t p) n -> p t n", p=P)
            )
            # M[:,0]=[a0;a1_0] M[:,1]=[a1_0;a0] M[:,2]=[a1_1;a1_2] M[:,3]=[a0;a1_1]
            M = am.tile([P, 4, CHUNK], f16, tag="M")
            nc.sync.dma_start(
                M[:], amixT_d[:, z0 : z0 + CHUNK].rearrange("(t p) n -> p t n", p=P)
            )

            # S[:,0]=S1=[t0;dt_0]  S[:,1]=S3=[t3_0;t4_0]  S[:,2]=S2=[dt_1;dt_2]
            # S[:,3]=S4'=[t4_1;t3_1]  S[:,4]=S5=[t3_2;t4_2]
            S = st.tile([P, 5, CHUNK], f16, tag="S")
            # S1 = [t0; dt_0]
            nc.vector.tensor_tensor(S[:, 0], F[:, 0], M[:, 0], MULT)
            # fused: S3 = T1 o M2, S2 = T2 o M3 in one FD-2048 op
            nc.gpsimd.tensor_tensor(S[:, 1:3], F[:, 0:2], M[:, 1:3], MULT)
            # S4' = [t4_1; t3_1] = [x1_1; x0] o [a0; a1_1]  (lhsT LA2)
            nc.vector.tensor_tensor(S[:, 3], F[:, 2], M[:, 3], MULT)
            # S5 = [t3_2; t4_2]: t3_2 = x0*a1_2 (out-base flex), t4_2 = x1_2*a0
            nc.vector.tensor_tensor(S[0:64, 4], F[64:128, 2], M[64:128, 2], MULT)
            nc.vector.tensor_tensor(S[64:128, 4], F[64:128, 1], M[64:128, 1], MULT)
            return S

        def crunch(ch, S):
            z0 = ch * CHUNK
            OT = ot.tile([P, 2, CHUNK], f16, tag="OT")
            B1 = ps_b.tile([P, CHUNK], f32, tag="B1")  # [s; g]
            # B23[:,0] = [o1_0; o1_1], B23[:,1] = [o1_2; s_dup]
            B23 = ps_b.tile([P, 2, CHUNK], f32, tag="B23")
            for h in range(2):
                hs = slice(h * 512, (h + 1) * 512)
                nc.tensor.matmul(B1[:, hs], Wc0[:, :], S[:, 0, hs], start=True, stop=False)
                nc.tensor.matmul(B1[:, hs], Wc4[:, :], S[:, 2, hs], start=False, stop=True)
                nc.tensor.matmul(B23[0:64, 0, hs], LA[:, :], S[:, 1, hs])
                nc.tensor.matmul(B23[64:128, 0, hs], LA2[:, :], S[:, 3, hs])
                nc.tensor.matmul(B23[0:64, 1, hs], LA[:, :], S[:, 4, hs])
                nc.tensor.matmul(
                    B23[64:128, 1, hs], Wc0[:, 0:64], S[:, 0, hs], start=True, stop=False
                )
                nc.tensor.matmul(
                    B23[64:128, 1, hs], Wc4[:, 0:64], S[:, 2, hs], start=False, stop=True
                )

            # U23[:,0] = [sg; sg], U23[:,1] = [sg; ss]
            U23 = usb.tile([P, 2, CHUNK], f16, tag="U23")
            nc.scalar.activation(U23[0:64, 0], B1[64:128, :], AF.Sigmoid)
            nc.scalar.copy(U23[64:128, 0], U23[0:64, 0])
            nc.scalar.activation(U23[64:128, 1], B23[64:128, 1], AF.Sigmoid)
            nc.scalar.copy(U23[0:64, 1], U23[0:64, 0])

            # gating (one FD-2048 op): OT[:,0]=[g0;g1], OT[:,1]=[g2;silu]
            nc.vector.tensor_tensor(OT[:, :], B23[:, :], U23[:, :], MULT)

            nc.sync.dma_start(
                outT_d[:, z0 : z0 + CHUNK].rearrange("(t p) n -> p t n", p=P),
                OT[:],
            )

        pend = {}
        for ch in range(N_CHUNKS + 1):
            if ch < N_CHUNKS:
                pend[ch] = prep(ch)
            if ch - 1 in pend:
                crunch(ch - 1, pend.pop(ch - 1))

        for pool in (ps_b, ot, usb, st, am, ft, wpool):
            pool.release()

    nc.compile()
    return nc


def _get_nc():
    global _BUILT
    if _BUILT is None:
        _BUILT = _build_nc()
    return _BUILT


def _host_prep(node_feats, node_attrs):
    """Feature-major fp16 layouts per core (layout/dtype/replication only)."""
    feats = np.ascontiguousarray(node_feats, dtype=np.float32)
    attrs = np.ascontiguousarray(node_attrs, dtype=np.float32)
    in_maps = []
    for c in range(N_CORES):
        f = feats[c * N_PER : (c + 1) * N_PER]
        a = attrs[c * N_PER : (c + 1) * N_PER].astype(np.float16)
        n = f.shape[0]
        x0 = f[:, :MUL]
        x1 = f[:, MUL:].reshape(n, MUL, 3)
        ftT = np.zeros((384, N_PAD), np.float16)
        ftT[0:64, :n] = x0.T
        ftT[64:128, :n] = x1[:, :, 0].T
        ftT[128:192, :n] = x1[:, :, 1].T
        ftT[192:256, :n] = x1[:, :, 2].T
        ftT[256:320, :n] = x1[:, :, 1].T
        ftT[320:384, :n] = x0.T
        amT = np.zeros((512, N_PAD), np.float16)
        a0, a10, a11, a12 = a[:, 0], a[:, 1], a[:, 2], a[:, 3]
        for r0, src in (
            (0, a0), (64, a10),      # M1 = [a0; a1_0]
            (128, a10), (192, a0),   # M2 = [a1_0; a0]
            (256, a11), (320, a12),  # M3 = [a1_1; a1_2]
            (384, a0), (448, a11),   # M4 = [a0; a1_1]
        ):
            amT[r0 : r0 + 64, :n] = src[None, :]
        in_maps.append({"featsT": ftT, "amixT": amT})
    return in_maps


def kernel(node_feats, node_attrs, W1, W2, W3, W4):
    from concourse.bass_utils import run_bass_kernel_spmd

    nc = _get_nc()
    in_maps = _host_prep(node_feats, node_attrs)
    for im in in_maps:
        im["W1"] = np.ascontiguousarray(W1, np.float32)
        im["W2"] = np.ascontiguousarray(W2, np.float32)
        im["W3"] = np.ascontiguousarray(W3, np.float32)
        im["W4"] = np.ascontiguousarray(W4, np.float32)

    res = run_bass_kernel_spmd(nc, in_maps, list(range(N_CORES)))
    global LAST_RESULT
    LAST_RESULT = res

    outs = []
    for c in range(N_CORES):
        oT = res.results[c]["outT"][:, :N_PER].astype(np.float32)  # [256, n]
        n = oT.shape[1]
        out = np.empty((n, 256), np.float32)
        out[:, :MUL] = oT[192:256, :].T  # silu(s)
        # gated rows: [g0(0:64) | g1(64:128) | g2(128:192)], col 64+3w+i
        out[:, MUL:] = oT[0:192, :].reshape(3, 64, n).transpose(2, 1, 0).reshape(n, 192)
        outs.append(out)
    return np.concatenate(outs, axis=0)


LAST_RESULT = None
